# revision 4
# baseline (speedup 1.0000x reference)
"""Multi-head causal attention on 8 TRN2 NeuronCores (v2, bf16).

Problem: x[4,2048,1024] @ Wqkv.T -> 16-head causal attention -> @ Wout.T.

Sharding: core c handles batch b=c//2, head-group g=c%2 (8 heads of 64).
Each core computes qkv for its (batch, head-group) slice, causal attention,
and a partial out-projection over its 512 columns of Wout's input dim.
Host sums the two partials per batch (the all-reduce of the hint).

v2 design (vs the fp32r baseline at ~441us):
- All matmul operands bf16 (host-cast, free). PSUM stays fp32.  FWL makes
  LDWEIGHTS ~4x cheaper so the weight loads hide behind streaming.
- Steady-state attention is ACT(exp)-bound per j-block (exp 1147ns vs
  ~650ns of PE work), so the Tensor queue is packed at emission time with
  "filler" work (next-chunk QKV production halves + prev-chunk out-proj
  tiles) popped from a deque whenever cumulative emitted-PE falls behind
  cumulative emitted-ACT.  need(key) force-drains the queue up to a
  producer before any consumer is emitted.
- Softmax normalization is off the PE critical path: ya/yb evacuate to
  SBUF right after the AV accumulation (freeing PSUM), then
  recip -> broadcast -> mul trail on DVE/GpSimd one m-segment later.
- Causal mask applied as a DVE multiply with 4 precomputed [128,1024]
  0/1 tiles instead of gpsimd affine_select on the critical path.
- V tiles pack [Ve|ones|0] and [0|ones|Vo] so the odd head's AV output
  lands at PSUM partitions 64:128 directly (no cross-partition pack-DMA);
  the ones columns produce the softmax denominators in the same matmul.
- DMA prologue: x chunk0 + wqk interleaved first, wv/wo after; chunk-0
  QK runs as 8 parallel PSUM accumulation chains so the PE starts ~1us in.
"""

import sys

sys.path.insert(0, "/opt/trn_rl_repo")

from collections import deque

import numpy as np

B, T, D, H = 4, 2048, 1024, 16
E = 512  # per-core head width (8 heads x 64)
ND = 8  # d chunks of 128
NTC = 4  # t chunks of 512
NBLK = 16  # 128-token key blocks
SCALE = 0.125  # 1/sqrt(64)

_NC_CACHE = {}


def build():
    if "nc" in _NC_CACHE:
        return _NC_CACHE["nc"]
    import concourse.bacc as bacc
    import concourse.mybir as mybir
    import concourse.tile as tile

    F32 = mybir.dt.float32
    BF16 = mybir.dt.bfloat16
    EXP = mybir.ActivationFunctionType.Exp

    nc = bacc.Bacc("TRN2", target_bir_lowering=False, debug=False, num_devices=8)
    xT = nc.declare_dram_parameter("xT", [D, T], BF16, isOutput=False)
    wqkT = nc.declare_dram_parameter("wqkT", [D, 2 * E], BF16, isOutput=False)
    wvT = nc.declare_dram_parameter("wvT", [D, E], BF16, isOutput=False)
    woT = nc.declare_dram_parameter("woT", [E, D], BF16, isOutput=False)
    z = nc.declare_dram_parameter("z", [T, D], F32, isOutput=True)

    from contextlib import ExitStack

    with tile.TileContext(nc) as tc:
        with ExitStack() as stack:
            pool_specs = {
                "pw": dict(bufs=8),
                "pwv": dict(bufs=8),
                "pwo": dict(bufs=4),
                "pqt": dict(bufs=1),
                "pkt": dict(bufs=1),
                "pvt": dict(bufs=1),
                "pmask": dict(bufs=4),
                "px": dict(bufs=16),
                "ppt": dict(bufs=3),
                "pysbr": dict(bufs=2),
                "pysbn": dict(bufs=8),
                "pdn": dict(bufs=2),
                "pdnl": dict(bufs=2),
                "prcp": dict(bufs=2),
                "prcpb": dict(bufs=2),
                "prb": dict(bufs=2),
                "pzsb": dict(bufs=2),
                "pst": dict(bufs=2, space="PSUM"),
                "pyd": dict(bufs=2, space="PSUM"),
                "pacc": dict(bufs=2, space="PSUM"),
            }
            P = {
                name: stack.enter_context(tc.tile_pool(name=name, **kw))
                for name, kw in pool_specs.items()
            }
            pw, pwv, pwo = P["pw"], P["pwv"], P["pwo"]
            pqt, pkt, pvt, pmask, px = (
                P["pqt"], P["pkt"], P["pvt"], P["pmask"], P["px"],
            )
            ppt, pysbr, pysbn, pdn, pdnl = (
                P["ppt"], P["pysbr"], P["pysbn"], P["pdn"], P["pdnl"],
            )
            prcp, prcpb, prb, pzsb = P["prcp"], P["prcpb"], P["prb"], P["pzsb"]
            pst, pyd, pacc = P["pst"], P["pyd"], P["pacc"]
            # ---------------- persistent tiles ----------------
            wqk = [pw.tile([128, 2 * E], BF16, tag="wqk", name="wqk") for _ in range(ND)]
            wv = [pwv.tile([128, E], BF16, tag="wv", name="wv") for _ in range(ND)]
            wo = [pwo.tile([128, D], BF16, tag="wo", name="wo") for _ in range(4)]
            QT = pqt.tile([128, NBLK * 512], BF16, tag="qt")
            KT = pkt.tile([128, 4 * T], BF16, tag="kt")
            VT = pvt.tile([128, NBLK * 1024], BF16, tag="vt")
            masks = [pmask.tile([128, 1024], BF16, tag="mask", name="mask") for _ in range(4)]

            # ---------------- DMA prologue ----------------
            xs = {0: [px.tile([128, 512], BF16, tag="x", name="xs") for _ in range(ND)]}
            for dc in range(ND):
                nc.sync.dma_start(xs[0][dc][:], xT[dc * 128 : (dc + 1) * 128, 0:512])
                nc.sync.dma_start(wqk[dc][:], wqkT[dc * 128 : (dc + 1) * 128, :])
            for dc in range(ND):
                nc.sync.dma_start(wv[dc][:], wvT[dc * 128 : (dc + 1) * 128, :])
            for m4 in range(4):
                nc.sync.dma_start(wo[m4][:], woT[m4 * 128 : (m4 + 1) * 128, :])

            # ---------------- constants ----------------
            # VT block layout per m (256 cols): [Ve(64)|1(32)|0(32)|0(32)|1(32)|Vo(64)]
            VT4 = VT[:].rearrange("p (b m s) -> p b m s", b=NBLK, m=4)
            nc.gpsimd.memset(VT4[:, :, :, 64:96], 1.0)
            nc.gpsimd.memset(VT4[:, :, :, 96:160], 0.0)
            nc.gpsimd.memset(VT4[:, :, :, 160:192], 1.0)
            # mask r: keep (p, c) iff c >= p + 128 r  (same pattern both halves)
            for r in range(4):
                nc.gpsimd.memset(masks[r][:], 1.0)
                for h in range(2):
                    half = masks[r][:, h * 512 : (h + 1) * 512]
                    nc.gpsimd.affine_select(
                        out=half,
                        in_=half,
                        compare_op=mybir.AluOpType.is_ge,
                        fill=0.0,
                        base=-128 * r,
                        pattern=[[1, 512]],
                        channel_multiplier=-1,
                    )

            # ---------------- emission bookkeeping ----------------
            cnt = {"pe": 0.0, "act": 0.0}
            MM = 240.0  # est ns per matmul issue slot (N=512)
            ACT_NS = 1150.0  # est ns per [128,1024] exp
            filler = deque()
            done = set()
            acc_live = {}

            def pump():
                key, fn = filler.popleft()
                fn()
                done.add(key)

            def need(key):
                while key not in done:
                    assert filler, f"need({key}) with empty filler"
                    pump()

            def fill_to_act():
                while filler and cnt["pe"] < cnt["act"]:
                    pump()

            # ---------------- worker emitters ----------------
            def qk_half(tcx, g, half):
                # g 0..3: Q pair m=g ; g 4..7: K pair m=g-4
                if half == 0:
                    acc_live[("qk", tcx, g)] = pacc.tile([128, 512], F32, tag="acc", name="acc")
                acc = acc_live[("qk", tcx, g)]
                for dc in range(half * 4, half * 4 + 4):
                    nc.tensor.matmul(
                        acc[:],
                        wqk[dc][:, g * 128 : (g + 1) * 128],
                        xs[tcx][dc][:],
                        start=(dc == 0),
                        stop=(dc == ND - 1),
                    )
                cnt["pe"] += 4 * MM
                if half == 1:
                    if g < 4:
                        q0 = (tcx * 4 + g) * 512
                        nc.vector.tensor_copy(QT[:, q0 : q0 + 512], acc[:])
                    else:
                        m = g - 4
                        k0 = m * T + tcx * 512
                        nc.vector.tensor_copy(KT[:, k0 : k0 + 512], acc[:])
                    del acc_live[("qk", tcx, g)]

            def v_half(tcx, ts, half):
                if half == 0:
                    acc_live[("v", tcx, ts)] = pacc.tile([128, 512], F32, tag="acc", name="acc")
                acc = acc_live[("v", tcx, ts)]
                for dc in range(half * 4, half * 4 + 4):
                    nc.tensor.matmul(
                        acc[:],
                        xs[tcx][dc][:, ts * 128 : (ts + 1) * 128],
                        wv[dc][:],
                        start=(dc == 0),
                        stop=(dc == ND - 1),
                    )
                cnt["pe"] += 4 * MM
                if half == 1:
                    blk = tcx * 4 + ts
                    a4 = acc[:].rearrange("p (m h c) -> p m h c", m=4, h=2)
                    vb = VT[:, blk * 1024 : (blk + 1) * 1024].rearrange(
                        "p (m h s) -> p m h s", m=4, h=2
                    )
                    nc.vector.tensor_copy(vb[:, :, 0:1, 0:64], a4[:, :, 0:1, :])
                    nc.vector.tensor_copy(vb[:, :, 1:2, 64:128], a4[:, :, 1:2, :])
                    del acc_live[("v", tcx, ts)]

            ysbn_t = {}

            def op_tile(tcx, ib, fh):
                zp = pacc.tile([128, 512], F32, tag="acc", name="acc")
                for m in range(4):
                    nc.tensor.matmul(
                        zp[:],
                        ysbn_t[(tcx, m)][:, ib * 128 : (ib + 1) * 128],
                        wo[m][:, fh * 512 : (fh + 1) * 512],
                        start=(m == 0),
                        stop=(m == 3),
                    )
                cnt["pe"] += 4 * MM
                zsb = pzsb.tile([128, 512], F32, tag="zsb", name="zsb")
                nc.vector.tensor_copy(zsb[:], zp[:])
                row = tcx * 512 + ib * 128
                nc.sync.dma_start(z[row : row + 128, fh * 512 : (fh + 1) * 512], zsb[:])

            # ---------------- prologue: chunk-0 QK as 8 parallel chains ----------------
            st_a = pst.tile([128, 1024], F32, tag="st", name="st")
            st_b = pst.tile([128, 1024], F32, tag="st", name="st")
            yd_a = pyd.tile([128, 512], F32, tag="yd", name="yd")
            yd_b = pyd.tile([128, 512], F32, tag="yd", name="yd")
            ac_a = pacc.tile([128, 512], F32, tag="acc", name="acc")
            ac_b = pacc.tile([128, 512], F32, tag="acc", name="acc")
            slots = [
                st_a[:, 0:512],
                st_a[:, 512:1024],
                st_b[:, 0:512],
                st_b[:, 512:1024],
                yd_a[:],
                yd_b[:],
                ac_a[:],
                ac_b[:],
            ]
            for dc in range(ND):
                for g in range(8):
                    nc.tensor.matmul(
                        slots[g],
                        wqk[dc][:, g * 128 : (g + 1) * 128],
                        xs[0][dc][:],
                        start=(dc == 0),
                        stop=(dc == ND - 1),
                    )
            cnt["pe"] += 64 * MM
            for g in range(8):
                if g < 4:
                    nc.vector.tensor_copy(QT[:, g * 512 : (g + 1) * 512], slots[g])
                else:
                    m = g - 4
                    nc.vector.tensor_copy(KT[:, m * T : m * T + 512], slots[g])
                done.add(("qk", 0, g, 0))
                done.add(("qk", 0, g, 1))
            for ts in range(4):
                for h in range(2):
                    filler.append(
                        (("v", 0, ts, h), lambda ts=ts, h=h: v_half(0, ts, h))
                    )

            # ---------------- attention driver ----------------
            norm_pending = []

            def flush_norms():
                while norm_pending:
                    norm_pending.pop(0)()

            for tcx in range(NTC):
                njb = 4 * tcx + 4
                if tcx + 1 < NTC:
                    xs[tcx + 1] = [
                        px.tile([128, 512], BF16, tag="x", name="xs") for _ in range(ND)
                    ]
                    t0 = (tcx + 1) * 512
                    for dc in range(ND):
                        nc.sync.dma_start(
                            xs[tcx + 1][dc][:],
                            xT[dc * 128 : (dc + 1) * 128, t0 : t0 + 512],
                        )
                for m in range(4):
                    flush_norms()
                    qoff = (tcx * 4 + m) * 512

                    def s_pair(jb, m=m, qoff=qoff):
                        st = pst.tile([128, 1024], F32, tag="st", name="st")
                        k0 = m * T + jb * 128
                        nc.tensor.matmul(
                            st[:, 0:512],
                            KT[0:64, k0 : k0 + 128],
                            QT[0:64, qoff : qoff + 512],
                            start=True,
                            stop=True,
                        )
                        nc.tensor.matmul(
                            st[:, 512:1024],
                            KT[64:128, k0 : k0 + 128],
                            QT[64:128, qoff : qoff + 512],
                            start=True,
                            stop=True,
                        )
                        cnt["pe"] += MM
                        return st

                    need(("qk", tcx, m, 1))  # qt for this (tc, m)
                    ya = pyd.tile([128, 512], F32, tag="yd", name="yd")
                    yb = pyd.tile([128, 512], F32, tag="yd", name="yd")
                    sts = [None] * njb
                    for jb in range(min(2, njb)):
                        need(("qk", jb // 4, 4 + m, 1))
                        sts[jb] = s_pair(jb)
                    for jb in range(njb):
                        st = sts[jb]
                        sts[jb] = None
                        pt = ppt.tile([128, 1024], BF16, tag="pt", name="pt")
                        nc.scalar.activation(pt[:], st[:], EXP, scale=SCALE)
                        cnt["act"] += ACT_NS
                        r = jb - 4 * tcx
                        if r >= 0:
                            nc.vector.tensor_mul(pt[:], pt[:], masks[r][:])
                        if jb + 2 < njb:
                            need(("qk", (jb + 2) // 4, 4 + m, 1))
                            sts[jb + 2] = s_pair(jb + 2)
                        need(("v", jb // 4, jb % 4, 1))
                        voff = jb * 1024 + m * 256
                        first, last = (jb == 0), (jb == njb - 1)
                        nc.tensor.matmul(
                            ya[:],
                            VT[:, voff : voff + 128],
                            pt[:, 0:512],
                            start=first,
                            stop=last,
                        )
                        nc.tensor.matmul(
                            yb[:],
                            VT[:, voff + 128 : voff + 256],
                            pt[:, 512:1024],
                            start=first,
                            stop=last,
                        )
                        cnt["pe"] += 2 * MM
                        fill_to_act()

                    # ---- m end: evacuate PSUM, defer normalize
                    ysbr = pysbr.tile([128, 512], BF16, tag="ysbr", name="ysbr")
                    nc.vector.tensor_copy(ysbr[0:64, :], ya[0:64, :])
                    nc.vector.tensor_copy(ysbr[64:128, :], yb[64:128, :])
                    dn = pdn.tile([128, 512], F32, tag="dn", name="dn")
                    nc.vector.tensor_copy(dn[64:65, :], ya[64:65, :])
                    nc.vector.tensor_copy(dn[32:33, :], yb[32:33, :])
                    dnl = pdnl.tile([1, 1024], F32, tag="dnl", name="dnl")
                    nc.sync.dma_start(dnl[0:1, 0:512], dn[64:65, :])
                    nc.sync.dma_start(dnl[0:1, 512:1024], dn[32:33, :])
                    ysbn = pysbn.tile([128, 512], BF16, tag="ysbn", name="ysbn")
                    ysbn_t[(tcx, m)] = ysbn

                    def norm(dnl=dnl, ysbr=ysbr, ysbn=ysbn):
                        rcp = prcp.tile([1, 1024], F32, tag="rcp", name="rcp")
                        nc.vector.reciprocal_approx_fast(rcp[0:1, :], dnl[0:1, :])
                        rcpb = prcpb.tile([1, 1024], BF16, tag="rcpb", name="rcpb")
                        nc.vector.tensor_copy(rcpb[0:1, :], rcp[0:1, :])
                        rb = prb.tile([128, 1024], BF16, tag="rb", name="rb")
                        nc.gpsimd.partition_broadcast(rb[:, :], rcpb[0:1, :])
                        nc.gpsimd.tensor_mul(
                            ysbn[0:64, :], ysbr[0:64, :], rb[0:64, 0:512]
                        )
                        nc.gpsimd.tensor_mul(
                            ysbn[64:128, :], ysbr[64:128, :], rb[64:128, 512:1024]
                        )

                    norm_pending.append(norm)

                # ---- tc end: queue next chunk producers + this chunk's oproj
                if tcx + 1 < NTC:
                    nx = tcx + 1
                    for g in (0, 4):
                        for h in range(2):
                            filler.append(
                                (
                                    ("qk", nx, g, h),
                                    lambda nx=nx, g=g, h=h: qk_half(nx, g, h),
                                )
                            )
                    for ts in range(4):
                        for h in range(2):
                            filler.append(
                                (
                                    ("v", nx, ts, h),
                                    lambda nx=nx, ts=ts, h=h: v_half(nx, ts, h),
                                )
                            )
                    for g in (1, 5):
                        for h in range(2):
                            filler.append(
                                (
                                    ("qk", nx, g, h),
                                    lambda nx=nx, g=g, h=h: qk_half(nx, g, h),
                                )
                            )
                    for ib in range(4):
                        for fh in range(2):
                            filler.append(
                                (
                                    ("op", tcx, ib, fh),
                                    lambda tcx=tcx, ib=ib, fh=fh: op_tile(
                                        tcx, ib, fh
                                    ),
                                )
                            )
                    for g in (2, 6, 3, 7):
                        for h in range(2):
                            filler.append(
                                (
                                    ("qk", nx, g, h),
                                    lambda nx=nx, g=g, h=h: qk_half(nx, g, h),
                                )
                            )
                else:
                    flush_norms()
                    for ib in range(4):
                        for fh in range(2):
                            filler.append(
                                (
                                    ("op", tcx, ib, fh),
                                    lambda tcx=tcx, ib=ib, fh=fh: op_tile(
                                        tcx, ib, fh
                                    ),
                                )
                            )
            flush_norms()
            while filler:
                pump()

    nc.finalize()
    _NC_CACHE["nc"] = nc
    return nc


def _in_maps(x, Wqkv, Wout):
    import ml_dtypes

    BF = ml_dtypes.bfloat16
    x = np.asarray(x, dtype=np.float32)
    Wqkv = np.asarray(Wqkv, dtype=np.float32)
    Wout = np.asarray(Wout, dtype=np.float32)
    xTs = [np.ascontiguousarray(x[b].T.astype(BF)) for b in range(B)]
    maps = []
    for c in range(8):
        b, g = divmod(c, 2)
        qrows = Wqkv[E * g : E * g + E]
        krows = Wqkv[D + E * g : D + E * g + E]
        vrows = Wqkv[2 * D + E * g : 2 * D + E * g + E]
        maps.append(
            {
                "xT": xTs[b],
                "wqkT": np.ascontiguousarray(
                    np.concatenate([qrows, krows], axis=0).T.astype(BF)
                ),
                "wvT": np.ascontiguousarray(vrows.T.astype(BF)),
                "woT": np.ascontiguousarray(Wout[:, E * g : E * g + E].T.astype(BF)),
            }
        )
    return maps


def _run(x, Wqkv, Wout, trace=False):
    from concourse.bass_utils import run_bass_kernel_spmd

    nc = build()
    res = run_bass_kernel_spmd(
        nc, _in_maps(x, Wqkv, Wout), core_ids=list(range(8)), trace=trace
    )
    out = np.empty((B, T, D), dtype=np.float32)
    for b in range(B):
        out[b] = res.results[2 * b]["z"] + res.results[2 * b + 1]["z"]
    return out, res


def kernel(x, Wqkv, Wout):
    out, _ = _run(x, Wqkv, Wout, trace=False)
    return out
